# revision 10
# baseline (speedup 1.0000x reference)
"""Trainium2 Bass kernel for DifferentiableSoftmaxTree NLL (hierarchical
softmax negative log-likelihood).

Math: the 2-way log_softmax at each tree node reduces to a softplus of a
logit difference, so for sample b with path nodes n_k / directions d_k:
    s_k  = features[b] . (node_weights[n_k,:,1] - node_weights[n_k,:,0])
    out[b] = sum_k mask_k * softplus((1-2 d_k) * s_k)

Strategy (data-parallel over batch, 8 cores x 512 samples, 4 blocks of 128):

  TOP LEVELS (0..8, heap ids 0..510): every sample visits all 9, so the
  PE matmuls the block's features against ALL 511 weight-diff columns
  (fp16, 4 contraction chunks into one PSUM bank). ACT copies the fp32
  PSUM logits to SBUF as fp16; one 2x-mode DVE multiply against a
  host-built signed multi-hot (sign at the 9 path nodes, 0 elsewhere)
  yields u = sign*logit at path nodes, exactly 0 off-path. softplus(0)=
  ln2 is constant, folded into a host-side per-sample correction that is
  applied to the result AFTER readback (keeps the device tail short).

  DEEP LEVELS (9..15): the HOST pre-gathers each sample's 7 deep rows
  with the path direction sign PRE-MULTIPLIED into the row (+-1 scaling
  of fp16 is exact), masked levels zeroed. Dot products on DVE: fp16
  multiply (2x), tree-fold 512->64 with 2x TT adds, then one small 3D
  reduce written straight into the u tile as fp16.

  SOFTPLUS: softplus = ln(1+e^u) via ACT Exp then Ln(x+1, accum_out=row
  sum).  This runtime's act_info.json has no softplus table, BUT the
  natural_log_exp_and_others set holds exp AND ln AND copy -- one
  ACT_TABLE_LOAD covers the whole kernel.  The stock table chooser picks
  the FIRST set containing each function (exp->exp_and_others,
  ln->natural_log: one load per switch, 4 per run), so _build_program
  patches the table map it feeds to insert_act_table_loads, emptying the
  sets before natural_log_exp_and_others (indices preserved -- the
  emitted act_func_set_id indexes the real act_info.json).

  DMA: two streams per block on the GpSimd HWDGE ring (so the triggers
  never queue behind the ACT table load): pda = deep rows + features
  (what DVE needs first), then pdb = featT + multi-hot (what the PE and
  the mask multiply need ~2us later). All triggers are issued up front;
  wtopT rides the Sync ring.

  (tensor_tensor_reduce wedges this runtime; gpsimd elementwise ops
  starve DVE via the shared SBUF ports -- both measured in a previous
  session, both avoided.)
"""

import numpy as np
from contextlib import ExitStack

import concourse.bass as bass
import concourse.mybir as mybir
import concourse.tile as tile
from concourse import bass_utils
import concourse.bacc as bacc

NUM_CLASSES = 50000
NUM_INTERNAL = NUM_CLASSES - 1
D = 512
B = 4096
K = 16
N_CORES = 8
BL = B // N_CORES          # samples per core
P = 128                    # partition dim
NBLK = BL // P             # 128-sample blocks per core
JTOP = 9                   # tree levels computed via PE matmul
NTOP = (1 << JTOP) - 1     # 511 heap nodes in levels 0..8
KD = K - JTOP              # 7 deep levels per sample
NU = NTOP + KD             # softplus terms per sample
LN2 = float(np.log(2.0))
FOLD_TO = 64               # level width after TT tree-folds

_AF = mybir.ActivationFunctionType
_OP = mybir.AluOpType
_F16 = mybir.dt.float16
_F32 = mybir.dt.float32

# pda per-sample row, fp16: deep signed weight rows then features
PDA_W = KD * D + D         # 4096
OF_FEAT = KD * D           # 3584
# pdb per-sample row, fp16: featT block rows then signed multi-hot (+pad)
PDB_W = D + NTOP + 1       # 1024
OF_MH = D                  # 512


_NLX = "natural_log_exp_and_others"


def _patch_act_tables():
    """Make insert_act_table_loads resolve every activation we use into
    the natural_log_exp_and_others set (one ACT_TABLE_LOAD total).

    The chooser takes the first set containing each function, so empty
    every set that precedes natural_log_exp_and_others.  Entry order (and
    hence act_func_set_id, the index into the real act_info.json) is
    unchanged; the kept set genuinely contains exp/ln/copy, so walrus's
    own lowering stays consistent.
    """
    import concourse.bacc as _bacc_mod

    real = _bacc_mod.get_activation_tables

    def patched(arch):
        tabs = {k: set(v) for k, v in real(arch).items()}
        seen_nlx = False
        for k in tabs:
            if k == _NLX:
                seen_nlx = True
            elif not seen_nlx:
                tabs[k] = set()
        return tabs

    _bacc_mod.get_activation_tables = patched
    return real


def _build_program():
    restore = _patch_act_tables()
    try:
        return _build_program_inner()
    finally:
        import concourse.bacc as _bacc_mod

        _bacc_mod.get_activation_tables = restore


def _build_program_inner():
    nc = bacc.Bacc(
        "TRN2",
        target_bir_lowering=False,
        debug=False,
        enable_asserts=False,
        num_devices=N_CORES,
    )
    pda_ap = nc.dram_tensor("pda", [BL, PDA_W], _F16, kind="ExternalInput").ap()
    pdb_ap = nc.dram_tensor("pdb", [BL, PDB_W], _F16, kind="ExternalInput").ap()
    wtopT_ap = nc.dram_tensor("wtopT", [P, 4 * NTOP], _F16, kind="ExternalInput").ap()
    # [partition, block] layout -- ONE tail DMA; the host untransposes.
    # Column NBLK holds the deep-part partial sum of the last block (its
    # softplus is split top/deep to shorten the device tail).
    out_ap = nc.dram_tensor("out", [P, NBLK + 1], _F32, kind="ExternalOutput").ap()

    with tile.TileContext(nc) as tc, ExitStack() as ctx:
        const_pool = ctx.enter_context(tc.tile_pool(name="const", bufs=1))
        pda_pool = ctx.enter_context(tc.tile_pool(name="pda", bufs=NBLK))
        pdb_pool = ctx.enter_context(tc.tile_pool(name="pdb", bufs=NBLK))
        lg_pool = ctx.enter_context(tc.tile_pool(name="lg", bufs=2))
        u_pool = ctx.enter_context(tc.tile_pool(name="u", bufs=2))
        e_pool = ctx.enter_context(tc.tile_pool(name="e", bufs=2))
        dump_pool = ctx.enter_context(tc.tile_pool(name="dump", bufs=2))
        small_pool = ctx.enter_context(tc.tile_pool(name="small", bufs=1))
        psum_pool = ctx.enter_context(tc.tile_pool(name="psum", bufs=2, space="PSUM"))

        # all input DMAs issued up front, ALL on the Sync ring (it enters
        # the main body ~0.7us before GpSimd and ~0.4us before Scalar),
        # strictly in need-order: the 16 HW DMA engines drain one queue
        # item at a time, so whatever is queued first lands first.  pda0
        # (deep rows, what DVE needs to start) goes first; wtopT after
        # pdb0 (the PE only needs it ~2us into block 0).
        pda_ts, pdb_ts = [], []
        for blk in range(NBLK):
            b0 = blk * P
            pda_t = pda_pool.tile([P, PDA_W], _F16, tag="pda")
            nc.sync.dma_start(pda_t[:], pda_ap[b0 : b0 + P, :])
            pda_ts.append(pda_t)
            pdb_t = pdb_pool.tile([P, PDB_W], _F16, tag="pdb")
            nc.sync.dma_start(pdb_t[:], pdb_ap[b0 : b0 + P, :])
            pdb_ts.append(pdb_t)
            if blk == 0:
                wt_t = const_pool.tile([P, 4 * NTOP], _F16, tag="wt")
                nc.sync.dma_start(wt_t[:], wtopT_ap[:])

        res_t = small_pool.tile([P, NBLK + 1], _F32, tag="res")

        for blk in range(NBLK):
            pda_t = pda_ts[blk]
            pdb_t = pdb_ts[blk]
            u_t = u_pool.tile([P, NU], _F16, tag="u")
            last = blk == NBLK - 1

            # all 511 top-level logits: featT.T @ wtopT -> PSUM
            ps_t = psum_pool.tile([P, NTOP], _F32, tag="ps")
            for c in range(4):
                nc.tensor.matmul(
                    ps_t[:],
                    lhsT=pdb_t[:, c * P : (c + 1) * P],
                    rhs=wt_t[:, c * NTOP : (c + 1) * NTOP],
                    start=(c == 0),
                    stop=(c == 3),
                )
            # ACT copies PSUM -> SBUF fp16 so the DVE multiply runs 2x
            lg_t = lg_pool.tile([P, NTOP], _F16, tag="lg")
            nc.scalar.activation(lg_t[:], ps_t[:], _AF.Copy)

            def emit_mh():
                nc.vector.tensor_tensor(
                    out=u_t[:, 0:NTOP],
                    in0=pdb_t[:, OF_MH : OF_MH + NTOP],
                    in1=lg_t[:],
                    op=_OP.mult,
                )

            # deep levels: in-place multiply by features, fold, reduce
            g3 = pda_t[:, 0 : KD * D].rearrange("p (k d) -> p k d", k=KD)
            nc.vector.tensor_tensor(
                out=g3,
                in0=g3,
                in1=pda_t[:, OF_FEAT : OF_FEAT + D][:, None, :].to_broadcast(
                    [P, KD, D]
                ),
                op=_OP.mult,
            )
            if last:
                # mask-mult goes FIRST (the 2us multiply above gives the
                # PE->Copy chain time to finish) so the 511-wide softplus
                # runs on ACT while DVE is still folding; only the 7-wide
                # deep softplus remains after the last DVE op.
                emit_mh()
                e_t = e_pool.tile([P, NTOP], _F32, tag="e")
                nc.scalar.activation(e_t[:], u_t[:, 0:NTOP], _AF.Exp)
                d_t = dump_pool.tile([P, NTOP], _F16, tag="d")
                nc.scalar.activation(
                    d_t[:], e_t[:], _AF.Ln, bias=1.0,
                    accum_out=res_t[:, blk : blk + 1],
                )
            w = D
            while w > FOLD_TO:
                h = w // 2
                nc.vector.tensor_tensor(
                    out=g3[:, :, 0:h], in0=g3[:, :, 0:h], in1=g3[:, :, h:w],
                    op=_OP.add,
                )
                w = h
            with nc.allow_low_precision("deep logits |s|<16; fp16 abs err ~4e-3"):
                nc.vector.tensor_reduce(
                    out=u_t[:, NTOP:NU], in_=g3[:, :, 0:FOLD_TO],
                    axis=mybir.AxisListType.X, op=_OP.add,
                )
            if last:
                ed_t = e_pool.tile([P, KD], _F32, tag="ed")
                nc.scalar.activation(ed_t[:], u_t[:, NTOP:NU], _AF.Exp)
                dd_t = dump_pool.tile([P, KD], _F16, tag="dd")
                nc.scalar.activation(
                    dd_t[:], ed_t[:], _AF.Ln, bias=1.0,
                    accum_out=res_t[:, NBLK : NBLK + 1],
                )
            else:
                emit_mh()
                # softplus row-sum: Exp then Ln(x+1, accum_out); the
                # accumulator IS the block result (host applies the ln2
                # correction after readback)
                e_t = e_pool.tile([P, NU], _F32, tag="e")
                nc.scalar.activation(e_t[:], u_t[:], _AF.Exp)
                d_t = dump_pool.tile([P, NU], _F16, tag="d")
                nc.scalar.activation(
                    d_t[:], e_t[:], _AF.Ln, bias=1.0,
                    accum_out=res_t[:, blk : blk + 1],
                )
        nc.sync.dma_start(out_ap[:], res_t[:])

    nc.compile()
    return nc


_PROGRAM_CACHE = {}


def _get_program():
    if "nc" not in _PROGRAM_CACHE:
        _PROGRAM_CACHE["nc"] = _build_program()
    return _PROGRAM_CACHE["nc"]


def _reset_device():
    # A previously-crashed kernel can leave an exec unit wedged; a
    # client-side axon reset clears it and is near-free otherwise.
    try:
        import ctypes

        lib = ctypes.CDLL("/opt/axon/libaxon_pjrt.so")
        lib.axon_reset.restype = ctypes.c_int64
        lib.axon_reset()
    except Exception:
        pass


def _prepare_inputs(features, targets, node_weights, path_nodes_map, path_directions_map):
    features = np.asarray(features, dtype=np.float32)
    targets = np.asarray(targets, dtype=np.int32)
    node_weights = np.asarray(node_weights, dtype=np.float32)
    path_nodes_map = np.asarray(path_nodes_map, dtype=np.int32)
    path_directions_map = np.asarray(path_directions_map, dtype=np.int32)

    wdiff = node_weights[:, :, 1] - node_weights[:, :, 0]     # [N_INT, D] f32
    maskmap = path_nodes_map != -1                             # [C, K]
    wdiff16 = wdiff.astype(np.float16)

    # top-level weight matrix, chunked for the PE:
    # wtopT[p, c*NTOP + n] = wdiff[n, c*128 + p]
    wtopT = np.ascontiguousarray(
        wdiff16[:NTOP].reshape(NTOP, 4, P).transpose(2, 1, 0).reshape(P, 4 * NTOP)
    )

    # per-sample metadata
    tflat = targets.reshape(-1)
    bnodes = path_nodes_map[tflat]                             # [B, K]
    bdirs = path_directions_map[tflat]
    bmask = maskmap[tflat]
    pathlen = bmask.sum(axis=1).astype(np.int32)               # 15 or 16
    sgn = (1 - 2 * bdirs).astype(np.float32)                   # [B, K]
    ncorr = -(NU - pathlen).astype(np.float32) * np.float32(LN2)

    # signed multi-hot over the 511 top nodes
    mh = np.zeros((B, NTOP + 1), dtype=np.float16)
    rows = np.arange(B)
    for j in range(JTOP):
        mh[rows, bnodes[:, j]] = sgn[:, j].astype(np.float16)

    # host pre-gather of each sample's deep-level rows, path sign folded
    # in (+-1 fp16 scaling is exact), masked levels zeroed
    deep_nodes = np.where(bmask[:, JTOP:], bnodes[:, JTOP:], 0)   # [B, KD]
    pdeep = wdiff16[deep_nodes]                                   # [B, KD, D]
    msgn = np.where(bmask[:, JTOP:], sgn[:, JTOP:], np.float32(0.0))
    pdeep *= msgn.astype(np.float16)[:, :, None]

    feat16 = features.astype(np.float16)                          # [B, D]

    in_maps = []
    for i in range(N_CORES):
        sl = slice(i * BL, (i + 1) * BL)
        fc = feat16[sl]                                           # [BL, D]
        # featT[blk*128+p, c*128+i] = fc[blk*128+i, c*128+p]
        ftT = fc.reshape(NBLK, P, 4, P).transpose(0, 3, 2, 1).reshape(BL, D)

        pda = np.empty((BL, PDA_W), dtype=np.float16)
        pda[:, 0 : KD * D] = pdeep[sl].reshape(BL, KD * D)
        pda[:, OF_FEAT : OF_FEAT + D] = fc
        pdb = np.empty((BL, PDB_W), dtype=np.float16)
        pdb[:, 0:D] = ftT
        pdb[:, OF_MH : OF_MH + NTOP + 1] = mh[sl]

        in_maps.append(
            {
                "pda": np.ascontiguousarray(pda),
                "pdb": np.ascontiguousarray(pdb),
                "wtopT": wtopT,
            }
        )
    return in_maps, ncorr


def kernel(features, targets, node_weights, path_nodes_map, path_directions_map):
    in_maps, ncorr = _prepare_inputs(
        features, targets, node_weights, path_nodes_map, path_directions_map
    )
    _reset_device()
    nc = _get_program()
    res = bass_utils.run_bass_kernel_spmd(nc, in_maps, core_ids=list(range(N_CORES)))
    # device output is [partition, block]; sample b = blk*128 + p.  The
    # device accumulates softplus over all 518 slots; off-path slots each
    # contribute softplus(0)=ln2, removed here via the host-built ncorr.
    outs = []
    for i in range(N_CORES):
        r = res.results[i]["out"]                  # [P, NBLK+1]
        r = r[:, :NBLK] + np.concatenate(
            [np.zeros((P, NBLK - 1), np.float32), r[:, NBLK:]], axis=1
        )
        outs.append(r.T.reshape(-1))
    out = np.concatenate(outs)
    return (out + ncorr).astype(np.float32)
